# Initial kernel scaffold
#
"""Trainium2 Bass kernel for the BNN/GLIF recurrent network (nn_BNNFC).

Strategy: 8-way tensor parallelism over the hidden dimension H=2048
(256 rows per core). The recurrence over T=512 steps is sequential; each
step does, per core:
  - syn psum = W_ix_shard @ x_t  +  W_hh_shard @ firing_full   (PE, bf16)
  - GLIF elementwise updates (DVE/ACT, f32 state)
  - firing shard -> AllGather across 8 cores -> full firing (h-major)
  - out_t[:, o_shard] = W_out_shard @ firing_full (PE) -> DRAM
Host side shards/transposes inputs and assembles the output.

Layouts (per core, h_local = 256 = 2 m-tiles of 128):
  global h = core*256 + ho*128 + p   (ho in {0,1}, p in [0,128))
  state tiles: [128(p), 2(ho), 64(b)] f32
  firing_full SBUF: [128(p), 8(core), 2(ho), 64(b)] bf16  (16 k-tiles)
"""
import sys, os, time
sys.path.insert(0, "/opt/trn_rl_repo")
import numpy as np

import concourse.bass as bass
import concourse.mybir as mybir
import concourse.tile as tile
from concourse import bacc
from concourse import bass_utils

F32 = mybir.dt.float32
BF16 = mybir.dt.bfloat16

IN, HID, OUT, A = 512, 2048, 512, 2
B, T = 64, 512
DT = 0.05
NC_N = 8            # cores
HL = HID // NC_N    # 256 h rows per core
HO = HL // 128      # 2 m-tiles
OL = OUT // NC_N    # 64 out features per core
NK = HID // 128     # 16 k-tiles over full H
NKI = IN // 128     # 4 k-tiles over input dim


def build(t_steps=T):
    nc = bacc.Bacc("TRN2", target_bir_lowering=False, debug=False,
                   num_devices=NC_N)

    # ---- external inputs (per-core values supplied via in_maps) ----
    xT_d = nc.dram_tensor("xT", [IN, t_steps, B], BF16, kind="ExternalInput")
    wix_d = nc.dram_tensor("wix", [128, NKI, HO, 128], BF16, kind="ExternalInput")
    whh_d = nc.dram_tensor("whh", [128, NK, HO, 128], BF16, kind="ExternalInput")
    wout_d = nc.dram_tensor("wout", [128, NK, OL], BF16, kind="ExternalInput")
    # per-partition param columns [128, HO] and broadcast tiles [128, HO, B]
    rcol_d = nc.dram_tensor("rcol", [A, 128, HO], F32, kind="ExternalInput")
    deccol_d = nc.dram_tensor("deccol", [A, 128, HO], F32, kind="ExternalInput")
    ampt_d = nc.dram_tensor("ampt", [A, 128, HO, B], F32, kind="ExternalInput")
    km1col_d = nc.dram_tensor("km1col", [128, HO], F32, kind="ExternalInput")
    negth_d = nc.dram_tensor("negth", [128, HO], F32, kind="ExternalInput")
    bivt_d = nc.dram_tensor("bivt", [128, HO, B], F32, kind="ExternalInput")
    boutcol_d = nc.dram_tensor("boutcol", [OL, 1], F32, kind="ExternalInput")

    out_d = nc.dram_tensor("out", [t_steps, OL, B], F32, kind="ExternalOutput")

    with tile.TileContext(nc) as tc:
        with (
            tc.tile_pool(name="static", bufs=1) as sp,
            tc.tile_pool(name="state", bufs=1) as st,
            tc.tile_pool(name="fire", bufs=2) as fp,
            tc.tile_pool(name="send", bufs=2) as sd,
            tc.tile_pool(name="xin", bufs=3) as xp,
            tc.tile_pool(name="tmp", bufs=2) as tp,
            tc.tile_pool(name="outs", bufs=2) as op_,
            tc.tile_pool(name="psyn", bufs=2, space="PSUM") as pps,
            tc.tile_pool(name="pout", bufs=2, space="PSUM") as ppo,
            tc.tile_pool(name="dram", bufs=2, space="DRAM") as dp,
        ):
            # ---- load static weights/params into SBUF ----
            wix = sp.tile([128, NKI, HO, 128], BF16)
            whh = sp.tile([128, NK, HO, 128], BF16)
            wout = sp.tile([128, NK, OL], BF16)
            rcol = sp.tile([A, 128, HO], F32)
            deccol = sp.tile([A, 128, HO], F32)
            ampt = sp.tile([A, 128, HO, B], F32)
            km1col = sp.tile([128, HO], F32)
            negth = sp.tile([128, HO], F32)
            bivt = sp.tile([128, HO, B], F32)
            boutcol = sp.tile([OL, 1], F32)
            nc.sync.dma_start(wix[:], wix_d[:])
            nc.sync.dma_start(whh[:], whh_d[:])
            nc.sync.dma_start(wout[:], wout_d[:])
            # params: first (A) dim is not the partition dim for rcol etc —
            # load each sub-tile separately so partition dim is 128.
            for a in range(A):
                nc.sync.dma_start(rcol[a], rcol_d[a])
                nc.sync.dma_start(deccol[a], deccol_d[a])
                nc.sync.dma_start(ampt[a], ampt_d[a])
            nc.sync.dma_start(km1col[:], km1col_d[:])
            nc.sync.dma_start(negth[:], negth_d[:])
            nc.sync.dma_start(bivt[:], bivt_d[:])
            nc.sync.dma_start(boutcol[:], boutcol_d[:])

            # ---- persistent state (f32), zero-init ----
            v = st.tile([128, HO, B], F32)
            a1 = st.tile([128, HO, B], F32)
            a2 = st.tile([128, HO, B], F32)
            fire32 = st.tile([128, HO, B], F32)   # own shard firing f32
            nc.vector.memset(v[:], 0.0)
            nc.vector.memset(a1[:], 0.0)
            nc.vector.memset(a2[:], 0.0)
            nc.vector.memset(fire32[:], 0.0)

            fire_prev = fp.tile([128, NC_N, HO, B], BF16, tag="fire")
            nc.gpsimd.memset(fire_prev[:], 0.0)

            for t in range(t_steps):
                # -- prefetch x_t (bf16 k-tiles) --
                xt = xp.tile([128, NKI, B], BF16, tag="xt")
                nc.sync.dma_start(xt[:], xT_d[:, :, t, :].transpose(1, 0, 2)
                                  if False else xT_d.ap().rearrange(
                                      "(ki p) tt b -> p ki tt b", p=128)[:, :, t, :])

                # -- syn matmuls into psum [128, HO, B] --
                psyn = pps.tile([128, HO, B], F32, tag="psyn")
                for ho in range(HO):
                    for ki in range(NKI):
                        nc.tensor.matmul(
                            psyn[:, ho, :], wix[:, ki, ho, :], xt[:, ki, :],
                            start=(ki == 0), stop=False)
                for kidx in range(NK):
                    for ho in range(HO):
                        nc.tensor.matmul(
                            psyn[:, ho, :], whh[:, kidx, ho, :],
                            fire_prev[:, kidx // HO, kidx % HO, :],
                            start=False, stop=(kidx == NK - 1))

                # -- GLIF elementwise --
                # off-critical-path pieces (depend only on prev state):
                u1 = tp.tile([128, HO, B], F32, tag="u1")
                u2 = tp.tile([128, HO, B], F32, tag="u2")
                g = tp.tile([128, HO, B], F32, tag="g")
                pre = tp.tile([128, HO, B], F32, tag="pre")
                for ho in range(HO):
                    # u_k = a_k * r_k + amp_k
                    nc.vector.scalar_tensor_tensor(
                        u1[:, ho, :], a1[:, ho, :], rcol[0, :, ho:ho + 1],
                        ampt[0, :, ho, :], mybir.AluOpType.mult,
                        mybir.AluOpType.add)
                    nc.vector.scalar_tensor_tensor(
                        u2[:, ho, :], a2[:, ho, :], rcol[1, :, ho:ho + 1],
                        ampt[1, :, ho, :], mybir.AluOpType.mult,
                        mybir.AluOpType.add)
                    # g = km1 - f   (km1 per-partition bias, scale=-1)
                    nc.scalar.activation(
                        g[:, ho, :], fire32[:, ho, :],
                        mybir.ActivationFunctionType.Identity,
                        bias=km1col[:, ho:ho + 1], scale=-1.0)
                # m_k = f * u_k ; a_k' = a_k*dec_k + m_k
                nc.vector.tensor_mul(u1[:], u1[:], fire32[:])
                nc.vector.tensor_mul(u2[:], u2[:], fire32[:])
                for ho in range(HO):
                    nc.vector.scalar_tensor_tensor(
                        a1[:, ho, :], a1[:, ho, :], deccol[0, :, ho:ho + 1],
                        u1[:, ho, :], mybir.AluOpType.mult, mybir.AluOpType.add)
                    nc.vector.scalar_tensor_tensor(
                        a2[:, ho, :], a2[:, ho, :], deccol[1, :, ho:ho + 1],
                        u2[:, ho, :], mybir.AluOpType.mult, mybir.AluOpType.add)
                # pre = a1' + a2' + v*g + b_iv
                nc.vector.tensor_mul(g[:], g[:], v[:])          # g <- v*(km1-f)
                nc.vector.tensor_add(pre[:], a1[:], a2[:])
                nc.vector.tensor_add(pre[:], pre[:], bivt[:])
                nc.vector.tensor_add(pre[:], pre[:], g[:])
                # critical: v = psum + pre ; f = sigmoid(v - thresh)
                nc.vector.tensor_add(v[:], psyn[:], pre[:])
                send = sd.tile([128, HO, B], BF16, tag="send")
                for ho in range(HO):
                    nc.scalar.activation(
                        send[:, ho, :], v[:, ho, :],
                        mybir.ActivationFunctionType.Sigmoid,
                        bias=negth[:, ho:ho + 1], scale=1.0)
                # f32 copy of firing for next step's elementwise
                nc.vector.tensor_copy(fire32[:], send[:])

                # -- allgather firing across cores --
                in_b = dp.tile([128, HO, B], BF16, tag="agin")
                out_b = dp.tile([NC_N, 128, HO, B], BF16, tag="agout")
                nc.sync.dma_start(in_b[:], send[:])
                nc.gpsimd.collective_compute(
                    "AllGather", mybir.AluOpType.bypass,
                    ins=[in_b[:].opt()], outs=[out_b[:].opt()],
                    replica_groups=[list(range(NC_N))])
                fire_new = fp.tile([128, NC_N, HO, B], BF16, tag="fire")
                half = NC_N // 2
                nc.sync.dma_start(
                    fire_new[:, :half, :, :],
                    out_b.ap().rearrange("c p ho b -> p c ho b")[:, :half, :, :])
                nc.sync.dma_start(
                    fire_new[:, half:, :, :],
                    out_b.ap().rearrange("c p ho b -> p c ho b")[:, half:, :, :])

                # -- output projection for this step (uses gathered fire_new) --
                pout = ppo.tile([OL, B], F32, tag="pout")
                for kidx in range(NK):
                    nc.tensor.matmul(
                        pout[:], wout[:, kidx, :],
                        fire_new[:, kidx // HO, kidx % HO, :],
                        start=(kidx == 0), stop=(kidx == NK - 1))
                osb = op_.tile([OL, B], F32, tag="osb")
                nc.scalar.activation(
                    osb[:], pout[:], mybir.ActivationFunctionType.Identity,
                    bias=boutcol[:], scale=1.0)
                nc.sync.dma_start(out_d[t], osb[:])

                fire_prev = fire_new

    nc.compile()
    return nc


# ---------------- host-side data prep ----------------
def prep_inputs(inp, W_iv, b_iv, W_out, b_out, thresh, k_m, asc_amp, asc_r,
                asc_k, t_steps=T):
    inp = np.asarray(inp, np.float32)
    W_iv = np.asarray(W_iv, np.float32)
    b_iv = np.asarray(b_iv, np.float32)
    W_out = np.asarray(W_out, np.float32)
    b_out = np.asarray(b_out, np.float32)
    thresh = np.asarray(thresh, np.float32).reshape(HID)
    k_m = np.asarray(k_m, np.float32).reshape(HID)
    asc_amp = np.asarray(asc_amp, np.float32).reshape(A, HID)
    asc_r = np.asarray(asc_r, np.float32).reshape(A, HID)
    asc_k = np.asarray(asc_k, np.float32).reshape(A, HID)

    W_ix = W_iv[:, :IN]          # (HID, IN)
    W_hh = W_iv[:, IN:]          # (HID, HID)
    dec = np.exp(np.float32(-DT) * asc_k).astype(np.float32)   # (A, HID)
    km1 = (1.0 - np.float32(DT) * k_m).astype(np.float32)      # (HID,)

    # xT: (IN, T, B) bf16  — same for all cores
    import ml_dtypes
    xT = np.ascontiguousarray(
        inp[:, :t_steps, :].transpose(2, 1, 0)).astype(ml_dtypes.bfloat16)

    in_maps = []
    for c in range(NC_N):
        hs = slice(c * HL, (c + 1) * HL)   # own h rows (global order!)
        # wix[p_k, ki, ho, p_m] = W_ix[c*HL + ho*128 + p_m, ki*128 + p_k]
        wix = W_ix[hs].reshape(HO, 128, NKI, 128).transpose(3, 2, 0, 1)
        whh = W_hh[hs].reshape(HO, 128, NK, 128).transpose(3, 2, 0, 1)
        os_ = slice(c * OL, (c + 1) * OL)
        wo = W_out[os_].reshape(OL, NK, 128).transpose(2, 1, 0)
        shp = lambda x: x[hs].reshape(HO, 128).T.copy()           # [128, HO]
        shpb = lambda x: np.repeat(
            x[hs].reshape(HO, 128).T[:, :, None], B, axis=2)      # [128,HO,B]
        m = {
            "xT": xT,
            "wix": np.ascontiguousarray(wix).astype(ml_dtypes.bfloat16),
            "whh": np.ascontiguousarray(whh).astype(ml_dtypes.bfloat16),
            "wout": np.ascontiguousarray(wo).astype(ml_dtypes.bfloat16),
            "rcol": np.stack([shp(asc_r[a]) for a in range(A)]),
            "deccol": np.stack([shp(dec[a]) for a in range(A)]),
            "ampt": np.stack([shpb(asc_amp[a]) for a in range(A)]),
            "km1col": shp(km1),
            "negth": shp(-thresh),
            "bivt": shpb(b_iv),
            "boutcol": b_out[os_].reshape(OL, 1).copy(),
        }
        in_maps.append(m)
    return in_maps


def assemble_output(results, t_steps=T):
    # per-core out: [t_steps, OL, B] -> full (B, T, OUT)
    full = np.empty((B, t_steps, OUT), np.float32)
    for c, r in enumerate(results):
        o = r["out"]                      # (t, OL, B)
        full[:, :, c * OL:(c + 1) * OL] = o.transpose(2, 0, 1)
    return full


_CACHE = {}


def _get_nc(t_steps=T):
    if t_steps not in _CACHE:
        _CACHE[t_steps] = build(t_steps)
    return _CACHE[t_steps]


def kernel(inp, W_iv, b_iv, W_out, b_out, thresh, k_m, asc_amp, asc_r, asc_k,
           t_steps=T):
    nc = _get_nc(t_steps)
    in_maps = prep_inputs(inp, W_iv, b_iv, W_out, b_out, thresh, k_m,
                          asc_amp, asc_r, asc_k, t_steps)
    res = bass_utils.run_bass_kernel_spmd(
        nc, in_maps, core_ids=list(range(NC_N)), trace=False)
    return assemble_output(res.results, t_steps)


# revision 9
# speedup vs baseline: 1.5443x; 1.5443x over previous
"""Trainium2 Bass kernel for the BNN/GLIF recurrent network (nn_BNNFC).

Strategy: 8-way tensor parallelism over the hidden dimension H=2048
(256 rows per core). The recurrence over T=512 steps is sequential; each
step does, per core:
  - syn psum = W_ix_shard @ x_t  +  W_hh_shard @ firing_full   (PE, bf16)
  - GLIF elementwise updates (DVE/ACT, f32 state)
  - firing shard -> AllGather across 8 cores -> full firing (h-major)
  - out_t[:, o_shard] = W_out_shard @ firing_full (PE) -> DRAM
Host side shards/transposes inputs and assembles the output.

Layouts (per core, h_local = 256 = 2 m-tiles of 128):
  global h = core*256 + ho*128 + p   (ho in {0,1}, p in [0,128))
  state tiles: [128(p), 2(ho), 64(b)] f32
  firing_full SBUF: [128(p), 8(core), 2(ho), 64(b)] bf16  (16 k-tiles)
"""
import sys, os, time
sys.path.insert(0, "/opt/trn_rl_repo")
import numpy as np

import concourse.bass as bass
import concourse.mybir as mybir
import concourse.tile as tile
from concourse import bacc
from concourse import bass_utils

F32 = mybir.dt.float32
BF16 = mybir.dt.bfloat16

IN, HID, OUT, A = 512, 2048, 512, 2
B, T = 64, 512
DT = 0.05
NC_N = 8            # cores
HL = HID // NC_N    # 256 h rows per core
HO = HL // 128      # 2 m-tiles
OL = OUT // NC_N    # 64 out features per core
NK = HID // 128     # 16 k-tiles over full H
NKI = IN // 128     # 4 k-tiles over input dim


def build(t_steps=T):
    nc = bacc.Bacc("TRN2", target_bir_lowering=False, debug=False,
                   num_devices=NC_N)

    # ---- external inputs (per-core values supplied via in_maps) ----
    xT_d = nc.dram_tensor("xT", [IN, t_steps, B], BF16, kind="ExternalInput")
    wix_d = nc.dram_tensor("wix", [128, NKI, HO, 128], BF16, kind="ExternalInput")
    whh_d = nc.dram_tensor("whh", [128, NK, HO, 128], BF16, kind="ExternalInput")
    wout_d = nc.dram_tensor("wout", [128, NK, OL], BF16, kind="ExternalInput")
    # per-partition param columns [128, HO] and broadcast tiles [128, HO, B]
    rcol_d = nc.dram_tensor("rcol", [128, A, HO], F32, kind="ExternalInput")
    deccol_d = nc.dram_tensor("deccol", [128, A, HO], F32, kind="ExternalInput")
    ampt_d = nc.dram_tensor("ampt", [128, A, HO, B], F32, kind="ExternalInput")
    km1t_d = nc.dram_tensor("km1t", [128, HO, B], F32, kind="ExternalInput")
    negth_d = nc.dram_tensor("negth", [128, HO], F32, kind="ExternalInput")
    bivt_d = nc.dram_tensor("bivt", [128, HO, B], F32, kind="ExternalInput")
    boutcol_d = nc.dram_tensor("boutcol", [OL, 1], F32, kind="ExternalInput")

    out_d = nc.dram_tensor("out", [t_steps, OL, B], F32, kind="ExternalOutput")

    with tile.TileContext(nc) as tc:
        with (
            tc.tile_pool(name="static", bufs=1) as sp,
            tc.tile_pool(name="state", bufs=1) as st,
            tc.tile_pool(name="fire", bufs=2) as fp,
            tc.tile_pool(name="send", bufs=2) as sd,
            tc.tile_pool(name="xin", bufs=3) as xp,
            tc.tile_pool(name="tmp", bufs=2) as tp,
            tc.tile_pool(name="outs", bufs=2) as op_,
            tc.tile_pool(name="psyn", bufs=2, space="PSUM") as pps,
            tc.tile_pool(name="pout", bufs=2, space="PSUM") as ppo,
            tc.tile_pool(name="dram", bufs=2, space="DRAM") as dp,
        ):
            # ---- load static weights/params into SBUF ----
            wix = sp.tile([128, NKI, HO, 128], BF16)
            whh = sp.tile([128, NK, HO, 128], BF16)
            wout = sp.tile([128, NK, OL], BF16)
            rcol = sp.tile([128, A, HO], F32)
            deccol = sp.tile([128, A, HO], F32)
            ampt = sp.tile([128, A, HO, B], F32)
            km1t = sp.tile([128, HO, B], F32)
            negth = sp.tile([128, HO], F32)
            bivt = sp.tile([128, HO, B], F32)
            boutcol = sp.tile([OL, 1], F32)
            nc.sync.dma_start(wix[:], wix_d[:])
            nc.sync.dma_start(whh[:], whh_d[:])
            nc.sync.dma_start(wout[:], wout_d[:])
            nc.sync.dma_start(rcol[:], rcol_d[:])
            nc.sync.dma_start(deccol[:], deccol_d[:])
            nc.sync.dma_start(ampt[:], ampt_d[:])
            nc.sync.dma_start(km1t[:], km1t_d[:])
            nc.sync.dma_start(negth[:], negth_d[:])
            nc.sync.dma_start(bivt[:], bivt_d[:])
            nc.sync.dma_start(boutcol[:], boutcol_d[:])

            # ---- persistent state (f32), zero-init ----
            v = st.tile([128, HO, B], F32)
            a1 = st.tile([128, HO, B], F32)
            a2 = st.tile([128, HO, B], F32)
            fire32 = st.tile([128, HO, B], F32)   # own shard firing f32
            nc.vector.memset(v[:], 0.0)
            nc.vector.memset(a1[:], 0.0)
            nc.vector.memset(a2[:], 0.0)
            nc.vector.memset(fire32[:], 0.0)

            fire_prev = fp.tile([128, NC_N, HO, B], BF16, tag="fire")
            nc.gpsimd.memset(fire_prev[:], 0.0)

            for t in range(t_steps):
                # -- prefetch x_t (bf16 k-tiles) --
                xt = xp.tile([128, NKI, B], BF16, tag="xt")
                nc.sync.dma_start(
                    xt[:],
                    xT_d.ap().rearrange("(ki p) tt b -> p ki tt b",
                                        p=128)[:, :, t, :])

                # -- syn matmuls into psum [128, HO, B] --
                psyn = pps.tile([128, HO, B], F32, tag="psyn")
                for ho in range(HO):
                    for ki in range(NKI):
                        nc.tensor.matmul(
                            psyn[:, ho, :], wix[:, ki, ho, :], xt[:, ki, :],
                            start=(ki == 0), stop=False)
                    for kidx in range(NK):
                        nc.tensor.matmul(
                            psyn[:, ho, :], whh[:, kidx, ho, :],
                            fire_prev[:, kidx // HO, kidx % HO, :],
                            start=False, stop=(kidx == NK - 1))

                # -- GLIF elementwise --
                # IEEE-faithful to the reference (inf/NaN propagation must
                # match: e.g. 0*inf=NaN in km1*v - f*v, so no factoring).
                # off-critical-path pieces (depend only on prev state):
                u1 = tp.tile([128, HO, B], F32, tag="u1")
                u2 = tp.tile([128, HO, B], F32, tag="u2")
                kv = tp.tile([128, HO, B], F32, tag="kv")
                fv = tp.tile([128, HO, B], F32, tag="fv")
                pre = tp.tile([128, HO, B], F32, tag="pre")
                for ho in range(HO):
                    # u_k = r_k * a_k + amp_k
                    nc.vector.scalar_tensor_tensor(
                        u1[:, ho, :], a1[:, ho, :], rcol[:, 0, ho:ho + 1],
                        ampt[:, 0, ho, :], mybir.AluOpType.mult,
                        mybir.AluOpType.add)
                    nc.vector.scalar_tensor_tensor(
                        u2[:, ho, :], a2[:, ho, :], rcol[:, 1, ho:ho + 1],
                        ampt[:, 1, ho, :], mybir.AluOpType.mult,
                        mybir.AluOpType.add)
                # m_k = f * u_k ; a_k' = a_k*dec_k + m_k
                nc.vector.tensor_mul(u1[:], u1[:], fire32[:])
                nc.vector.tensor_mul(u2[:], u2[:], fire32[:])
                for ho in range(HO):
                    nc.vector.scalar_tensor_tensor(
                        a1[:, ho, :], a1[:, ho, :], deccol[:, 0, ho:ho + 1],
                        u1[:, ho, :], mybir.AluOpType.mult, mybir.AluOpType.add)
                    nc.vector.scalar_tensor_tensor(
                        a2[:, ho, :], a2[:, ho, :], deccol[:, 1, ho:ho + 1],
                        u2[:, ho, :], mybir.AluOpType.mult, mybir.AluOpType.add)
                # pre = a1' + a2' + b_iv + km1*v - f*v  (unfactored)
                nc.vector.tensor_mul(kv[:], v[:], km1t[:])
                nc.vector.tensor_mul(fv[:], v[:], fire32[:])
                nc.vector.tensor_add(pre[:], a1[:], a2[:])
                nc.vector.tensor_add(pre[:], pre[:], bivt[:])
                nc.vector.tensor_add(pre[:], pre[:], kv[:])
                nc.vector.tensor_tensor(
                    pre[:], pre[:], fv[:], mybir.AluOpType.subtract)
                # critical: v = psum + pre ; f = sigmoid(v - thresh)
                nc.vector.tensor_add(v[:], psyn[:], pre[:])
                send = sd.tile([128, HO, B], BF16, tag="send")
                for ho in range(HO):
                    nc.scalar.activation(
                        send[:, ho, :], v[:, ho, :],
                        mybir.ActivationFunctionType.Sigmoid,
                        bias=negth[:, ho:ho + 1], scale=1.0)
                # f32 firing for next step's state math (off critical path)
                for ho in range(HO):
                    nc.scalar.activation(
                        fire32[:, ho, :], v[:, ho, :],
                        mybir.ActivationFunctionType.Sigmoid,
                        bias=negth[:, ho:ho + 1], scale=1.0)

                # -- allgather firing across cores --
                in_b = dp.tile([128, HO, B], BF16, tag="agin")
                out_b = dp.tile([NC_N, 128, HO, B], BF16, tag="agout")
                nc.sync.dma_start(in_b[:], send[:])
                nc.gpsimd.collective_compute(
                    "AllGather", mybir.AluOpType.bypass,
                    ins=[in_b[:].opt()], outs=[out_b[:].opt()],
                    replica_groups=[list(range(NC_N))])
                fire_new = fp.tile([128, NC_N, HO, B], BF16, tag="fire")
                half = NC_N // 2
                nc.sync.dma_start(
                    fire_new[:, :half, :, :],
                    out_b[:].rearrange("c p ho b -> p c ho b")[:, :half, :, :])
                nc.sync.dma_start(
                    fire_new[:, half:, :, :],
                    out_b[:].rearrange("c p ho b -> p c ho b")[:, half:, :, :])

                # -- output projection for this step (uses gathered fire_new) --
                pout = ppo.tile([OL, B], F32, tag="pout")
                for kidx in range(NK):
                    nc.tensor.matmul(
                        pout[:], wout[:, kidx, :],
                        fire_new[:, kidx // HO, kidx % HO, :],
                        start=(kidx == 0), stop=(kidx == NK - 1))
                osb = op_.tile([OL, B], F32, tag="osb")
                nc.scalar.activation(
                    osb[:], pout[:], mybir.ActivationFunctionType.Identity,
                    bias=boutcol[:], scale=1.0)
                nc.sync.dma_start(out_d[t], osb[:])

                fire_prev = fire_new

    nc.compile()
    return nc


# ---------------- host-side data prep ----------------
def prep_inputs(inp, W_iv, b_iv, W_out, b_out, thresh, k_m, asc_amp, asc_r,
                asc_k, t_steps=T):
    inp = np.asarray(inp, np.float32)
    W_iv = np.asarray(W_iv, np.float32)
    b_iv = np.asarray(b_iv, np.float32)
    W_out = np.asarray(W_out, np.float32)
    b_out = np.asarray(b_out, np.float32)
    thresh = np.asarray(thresh, np.float32).reshape(HID)
    k_m = np.asarray(k_m, np.float32).reshape(HID)
    asc_amp = np.asarray(asc_amp, np.float32).reshape(A, HID)
    asc_r = np.asarray(asc_r, np.float32).reshape(A, HID)
    asc_k = np.asarray(asc_k, np.float32).reshape(A, HID)

    W_ix = W_iv[:, :IN]          # (HID, IN)
    W_hh = W_iv[:, IN:]          # (HID, HID)
    dec = np.exp(np.float32(-DT) * asc_k).astype(np.float32)   # (A, HID)
    km1 = (1.0 - np.float32(DT) * k_m).astype(np.float32)      # (HID,)

    # xT: (IN, T, B) bf16  — same for all cores
    import ml_dtypes
    xT = np.ascontiguousarray(
        inp[:, :t_steps, :].transpose(2, 1, 0)).astype(ml_dtypes.bfloat16)

    in_maps = []
    for c in range(NC_N):
        hs = slice(c * HL, (c + 1) * HL)   # own h rows (global order!)
        # wix[p_k, ki, ho, p_m] = W_ix[c*HL + ho*128 + p_m, ki*128 + p_k]
        wix = W_ix[hs].reshape(HO, 128, NKI, 128).transpose(3, 2, 0, 1)
        whh = W_hh[hs].reshape(HO, 128, NK, 128).transpose(3, 2, 0, 1)
        os_ = slice(c * OL, (c + 1) * OL)
        wo = W_out[os_].reshape(OL, NK, 128).transpose(2, 1, 0)
        shp = lambda x: x[hs].reshape(HO, 128).T.copy()           # [128, HO]
        shpb = lambda x: np.repeat(
            x[hs].reshape(HO, 128).T[:, :, None], B, axis=2)      # [128,HO,B]
        m = {
            "xT": xT,
            "wix": np.ascontiguousarray(wix).astype(ml_dtypes.bfloat16),
            "whh": np.ascontiguousarray(whh).astype(ml_dtypes.bfloat16),
            "wout": np.ascontiguousarray(wo).astype(ml_dtypes.bfloat16),
            "rcol": np.stack([shp(asc_r[a]) for a in range(A)], axis=1).copy(),
            "deccol": np.stack([shp(dec[a]) for a in range(A)], axis=1).copy(),
            "ampt": np.stack([shpb(asc_amp[a]) for a in range(A)], axis=1).copy(),
            "km1t": shpb(km1),
            "negth": shp(-thresh),
            "bivt": shpb(b_iv),
            "boutcol": b_out[os_].reshape(OL, 1).copy(),
        }
        in_maps.append(m)
    return in_maps


def assemble_output(results, t_steps=T):
    # per-core out: [t_steps, OL, B] -> full (B, T, OUT)
    full = np.empty((B, t_steps, OUT), np.float32)
    for c, r in enumerate(results):
        o = r["out"]                      # (t, OL, B)
        full[:, :, c * OL:(c + 1) * OL] = o.transpose(2, 0, 1)
    return full


_CACHE = {}


def _get_nc(t_steps=T):
    if t_steps not in _CACHE:
        _CACHE[t_steps] = build(t_steps)
    return _CACHE[t_steps]


def kernel(inp, W_iv, b_iv, W_out, b_out, thresh, k_m, asc_amp, asc_r, asc_k,
           t_steps=T):
    nc = _get_nc(t_steps)
    in_maps = prep_inputs(inp, W_iv, b_iv, W_out, b_out, thresh, k_m,
                          asc_amp, asc_r, asc_k, t_steps)
    res = bass_utils.run_bass_kernel_spmd(
        nc, in_maps, core_ids=list(range(NC_N)), trace=False)
    return assemble_output(res.results, t_steps)
